# revision 9
# baseline (speedup 1.0000x reference)
"""MixtureAttention (MoE attention routing) Trainium2 kernel.

Strategy: expert-parallel over 8 NeuronCores (one expert per core).
Each core computes its expert's full attention output for all tokens,
multiplies by the per-token router weight (0 for tokens that did not
select this expert in their top-2), and the host sums the 8 per-core
outputs — the top-k combine becomes a plain sum because the router
weight is zero for non-selected experts.

Router math on device (per token): top-2 of 8 logits == (m1, m2) the
two largest logits; softmax over them gives w1 = sigmoid(m1-m2),
w2 = 1-w1; this core's weight is w1/w2/0 by comparing its own logit
against m1/m2 (exact float equality; ties are measure-zero).

Layout: activations are kept transposed ("T-layout", feature on
partitions, token on free dim) so every matmul contracts along
partitions.  Per (batch, 512-token chunk):
  qT (PE transpose) -> Q = wq^T qT (+bq, *hd^-0.5) -> per head:
  S^T[mk,nq] = K_h^T(lhsT) @ Q_h -> exp (ACT, no max needed: |S|<~3)
  -> AV with a ones-column appended to V (65th column) so the softmax
  denominator lands in psum row 64 -> reciprocal + PE-ones broadcast
  -> normalize -> O-proj (+bo, *router weight) -> out[b, d, token].

All big matmuls run in float32r (TF32-like, ~1e-4 rel err, 4x faster
than fp32 on the PE).  Everything else is fp32.
"""

import numpy as np

B, N, D, E, H = 2, 2048, 1024, 8, 16
MK = 512            # keys/values chunk per expert (M // E)
HD = D // H         # 64
P = 128
KO = D // P         # 8
NQC = 512           # token chunk (matmul free dim)
NCH = N // NQC      # 4
SCALE = HD ** -0.5
CORES = 8

_NC = None


def _build_nc():
    import concourse.bacc as bacc
    import concourse.mybir as mybir
    from concourse.tile import TileContext
    from concourse.masks import make_identity

    f32 = mybir.dt.float32
    f32r = mybir.dt.float32r
    Af = mybir.ActivationFunctionType
    Op = mybir.AluOpType

    nc = bacc.Bacc("TRN2", target_bir_lowering=False)

    q_d = nc.declare_dram_parameter("q", [B, N, D], f32, isOutput=False)
    k_d = nc.declare_dram_parameter("k", [B, MK, D], f32, isOutput=False)
    v_d = nc.declare_dram_parameter("v", [B, MK, D], f32, isOutput=False)
    wq_d = nc.declare_dram_parameter("wq", [D, D], f32r, isOutput=False)
    wk_d = nc.declare_dram_parameter("wk", [D, D], f32r, isOutput=False)
    wv_d = nc.declare_dram_parameter("wv", [D, D], f32r, isOutput=False)
    wo_d = nc.declare_dram_parameter("wo", [D, D], f32r, isOutput=False)
    bq_d = nc.declare_dram_parameter("bq", [D], f32, isOutput=False)
    bk_d = nc.declare_dram_parameter("bk", [D], f32, isOutput=False)
    bv_d = nc.declare_dram_parameter("bv", [D], f32, isOutput=False)
    bo_d = nc.declare_dram_parameter("bo", [D], f32, isOutput=False)
    # wr: [Wr | Wr[:, e]] so column 8 is this core's own-expert logit
    wr_d = nc.declare_dram_parameter("wr", [D, E + 1], f32, isOutput=False)
    br_d = nc.declare_dram_parameter("br", [E + 1], f32, isOutput=False)
    o_d = nc.declare_dram_parameter("o", [B, D, N], f32, isOutput=True)

    wq_r = wq_d.rearrange("(ki p) o -> p ki o", p=P)
    wk_r = wk_d.rearrange("(ki p) o -> p ki o", p=P)
    wv_r = wv_d.rearrange("(ki p) o -> p ki o", p=P)
    wo_r = wo_d.rearrange("(ki p) o -> p ki o", p=P)
    wr_r = wr_d.rearrange("(ki p) e -> p ki e", p=P)

    import concourse.bass as bass

    def pbcast(ap, nparts):
        # partition-stride-0 DMA source: replicate a [..] dram vector to
        # nparts partitions
        return bass.AP(tensor=ap.tensor, offset=ap.offset,
                       ap=[[0, nparts]] + list(ap.ap))

    with TileContext(nc) as tc:
        with tc.tile_pool(name="const", bufs=1) as cst, \
             tc.tile_pool(name="kvlong", bufs=1) as kvl, \
             tc.tile_pool(name="psp", bufs=1, space="PSUM") as psp:

            ident = cst.tile([P, P], f32, tag="ident")
            make_identity(nc, ident[:])
            ones32 = cst.tile([P, P], f32, tag="ones32")
            nc.vector.memset(ones32[:], 1.0)
            ones_r = cst.tile([P, P], f32r, tag="ones_r")
            nc.vector.tensor_copy(ones_r[:], ones32[:])

            wq_sb = cst.tile([P, KO, D], f32r, tag="wq")
            wo_sb = cst.tile([P, KO, D], f32r, tag="wo")
            for ki in range(KO):
                nc.sync.dma_start(wq_sb[:, ki], wq_r[:, ki])
                nc.sync.dma_start(wo_sb[:, ki], wo_r[:, ki])
            wr_sb = cst.tile([P, KO, E + 1], f32, tag="wr")
            nc.sync.dma_start(wr_sb[:], wr_r[:])

            bq_sb = cst.tile([P, KO], f32, tag="bq")
            bk_sb = cst.tile([P, KO], f32, tag="bk")
            bo_sb = cst.tile([P, KO], f32, tag="bo")
            nc.sync.dma_start(bq_sb[:], bq_d.rearrange("(ko p) -> p ko", p=P))
            nc.sync.dma_start(bk_sb[:], bk_d.rearrange("(ko p) -> p ko", p=P))
            nc.sync.dma_start(bo_sb[:], bo_d.rearrange("(ko p) -> p ko", p=P))
            bv_bc = cst.tile([P, D], f32, tag="bv")
            nc.gpsimd.dma_start(bv_bc[:], pbcast(bv_d[:], P))
            br_bc = cst.tile([P, E + 1], f32, tag="br")
            nc.gpsimd.dma_start(br_bc[:], pbcast(br_d[:], P))

            KT = kvl.tile([P, KO, MK], f32r, tag="KT")
            V = kvl.tile([P, MK // P, H * (HD + 1)], f32r, tag="V")

            for b in range(B):
                # ---- K/V setup for this batch ----
                with tc.tile_pool(name="setup", bufs=1) as stp, \
                     tc.tile_pool(name="setups", bufs=2) as stps:
                    kT = stp.tile([P, KO, MK], f32r, tag="kT")
                    vT = stp.tile([P, KO, MK], f32r, tag="vT")
                    for src, dst in ((k_d, kT), (v_d, vT)):
                        for tt in range(MK // P):
                            xn = stps.tile([P, D], f32, tag="xn")
                            nc.sync.dma_start(xn[:], src[b, tt * P:(tt + 1) * P])
                            for ko in range(KO):
                                pt = psp.tile([P, P], f32, tag="sm", bufs=2)
                                nc.tensor.transpose(
                                    pt[:], xn[:, ko * P:(ko + 1) * P], ident[:])
                                nc.vector.tensor_copy(
                                    dst[:, ko, tt * P:(tt + 1) * P], pt[:])
                    # KT = wk^T @ kT + bk   (dout on partitions, mk free)
                    for ko in range(KO):
                        wkt = stps.tile([P, KO, P], f32r, tag="wkt")
                        nc.sync.dma_start(wkt[:], wk_r[:, :, ko * P:(ko + 1) * P])
                        pk = psp.tile([P, MK], f32, tag="big", bufs=4)
                        for ki in range(KO):
                            nc.tensor.matmul(pk[:], wkt[:, ki], kT[:, ki],
                                             start=(ki == 0), stop=(ki == KO - 1))
                        nc.vector.tensor_scalar(
                            KT[:, ko], pk[:], bk_sb[:, ko:ko + 1], None, Op.add)
                    # V natural [mk, dout] = vT^T @ wv + bv, interleaved with a
                    # ones column every HD+1 so AV also produces the softmax sum
                    vview = V[:].rearrange("p m (h c) -> p m h c", c=HD + 1)
                    nc.vector.tensor_copy(
                        vview[:, :, :, HD],
                        ones32[:, :(MK // P) * H].rearrange(
                            "p (m h) -> p m h", m=MK // P))
                    for half in range(2):
                        wvt = stp.tile([P, KO, D // 2], f32r, tag="wvt")
                        nc.sync.dma_start(
                            wvt[:], wv_r[:, :, half * (D // 2):(half + 1) * (D // 2)])
                        for mt in range(MK // P):
                            pv = psp.tile([P, D // 2], f32, tag="big", bufs=4)
                            for ki in range(KO):
                                nc.tensor.matmul(
                                    pv[:], vT[:, ki, mt * P:(mt + 1) * P],
                                    wvt[:, ki],
                                    start=(ki == 0), stop=(ki == KO - 1))
                            hsl = slice(half * (H // 2), (half + 1) * (H // 2))
                            nc.vector.tensor_tensor(
                                vview[:, mt, hsl, :HD],
                                pv[:].rearrange("p (h c) -> p h c", c=HD),
                                bv_bc[:, half * (D // 2):(half + 1) * (D // 2)]
                                .rearrange("p (h c) -> p h c", c=HD),
                                Op.add)

                # ---- chunk loop ----
                with tc.tile_pool(name="chunk", bufs=1) as chk, \
                     tc.tile_pool(name="chks", bufs=2) as chs, \
                     tc.tile_pool(name="pt_pool", bufs=4) as ptp, \
                     tc.tile_pool(name="fin_pool", bufs=3) as fpl:
                    for c in range(NCH):
                        tok0 = c * NQC
                        qTc = chk.tile([P, KO, NQC], f32r, tag="qTc")
                        for tt in range(NQC // P):
                            qn = chs.tile([P, D], f32, tag="qn")
                            nc.sync.dma_start(
                                qn[:], q_d[b, tok0 + tt * P: tok0 + (tt + 1) * P])
                            for ko in range(KO):
                                pt = psp.tile([P, P], f32, tag="sm", bufs=2)
                                nc.tensor.transpose(
                                    pt[:], qn[:, ko * P:(ko + 1) * P], ident[:])
                                nc.vector.tensor_copy(
                                    qTc[:, ko, tt * P:(tt + 1) * P], pt[:])

                        # ---- router ----
                        Lg = chs.tile([P, NQC // P, E + 1], f32, tag="Lg")
                        for tt in range(NQC // P):
                            pr = psp.tile([P, E + 1], f32, tag="sm", bufs=2)
                            for ki in range(KO):
                                nc.tensor.matmul(
                                    pr[:],
                                    qTc[:, ki, tt * P:(tt + 1) * P].bitcast(f32),
                                    wr_sb[:, ki],
                                    start=(ki == 0), stop=(ki == KO - 1))
                            nc.vector.tensor_tensor(Lg[:, tt], pr[:], br_bc[:],
                                                    Op.add)
                        m1 = chs.tile([P, NQC // P], f32, tag="m1")
                        m2 = chs.tile([P, NQC // P], f32, tag="m2")
                        msk = chs.tile([P, NQC // P, E], f32, tag="msk")
                        nc.vector.tensor_reduce(m1[:], Lg[:, :, :E],
                                                mybir.AxisListType.X, Op.max)
                        nc.vector.tensor_tensor(
                            msk[:], Lg[:, :, :E],
                            m1[:, :, None].to_broadcast((P, NQC // P, E)),
                            Op.is_equal)
                        nc.vector.tensor_scalar(msk[:], msk[:], -1e30, None,
                                                Op.mult)
                        nc.vector.tensor_tensor(msk[:], Lg[:, :, :E], msk[:],
                                                Op.add)
                        nc.vector.tensor_reduce(m2[:], msk[:],
                                                mybir.AxisListType.X, Op.max)
                        dd = chs.tile([P, NQC // P], f32, tag="dd")
                        w1 = chs.tile([P, NQC // P], f32, tag="w1")
                        nc.vector.tensor_tensor(dd[:], m1[:], m2[:], Op.subtract)
                        nc.scalar.activation(w1[:], dd[:], Af.Sigmoid)
                        eq1 = chs.tile([P, NQC // P], f32, tag="eq1")
                        eq2 = chs.tile([P, NQC // P], f32, tag="eq2")
                        we = chs.tile([P, NQC // P], f32, tag="we")
                        nc.vector.tensor_tensor(eq1[:], Lg[:, :, E], m1[:],
                                                Op.is_equal)
                        nc.vector.tensor_tensor(eq2[:], Lg[:, :, E], m2[:],
                                                Op.is_equal)
                        nc.vector.tensor_tensor(eq1[:], eq1[:], w1[:], Op.mult)
                        # w2 = 1 - w1
                        nc.vector.tensor_scalar(w1[:], w1[:], -1.0, 1.0,
                                                Op.mult, Op.add)
                        nc.vector.tensor_tensor(eq2[:], eq2[:], w1[:], Op.mult)
                        nc.vector.tensor_tensor(we[:], eq1[:], eq2[:], Op.add)
                        wrow = chs.tile([1, NQC], f32r, tag="wrow")
                        for tt in range(NQC // P):
                            pw = psp.tile([1, P], f32, tag="sm", bufs=2)
                            nc.tensor.transpose(pw[:], we[:, tt:tt + 1],
                                                ident[:])
                            nc.vector.tensor_copy(
                                wrow[0:1, tt * P:(tt + 1) * P], pw[0:1, :])
                        pwb = psp.tile([P, NQC], f32, tag="big", bufs=4)
                        nc.tensor.matmul(pwb[:], ones_r[0:1, :P], wrow[0:1, :],
                                         start=True, stop=True)
                        w_sb = chs.tile([P, NQC], f32, tag="w_sb")
                        nc.scalar.copy(w_sb[:], pwb[:])

                        # ---- Q projection (scale folded in) ----
                        Qc = chk.tile([P, KO, NQC], f32r, tag="Qc")
                        for ko in range(KO):
                            pq = psp.tile([P, NQC], f32, tag="big", bufs=4)
                            for ki in range(KO):
                                nc.tensor.matmul(
                                    pq[:], wq_sb[:, ki, ko * P:(ko + 1) * P],
                                    qTc[:, ki],
                                    start=(ki == 0), stop=(ki == KO - 1))
                            nc.vector.tensor_scalar(
                                Qc[:, ko], pq[:], bq_sb[:, ko:ko + 1], SCALE,
                                Op.add, Op.mult)

                        # ---- heads ----
                        O_sb = chk.tile([P, KO, NQC], f32r, tag="O_sb")
                        for h in range(H):
                            p0 = (h % 2) * HD
                            koh = h // 2
                            po = psp.tile([HD + 1, NQC], f32, tag="po", bufs=2)
                            for mt in range(MK // P):
                                ps = psp.tile([P, NQC], f32, tag="big", bufs=4)
                                nc.tensor.matmul(
                                    ps[:],
                                    KT[p0:p0 + HD, koh, mt * P:(mt + 1) * P],
                                    Qc[p0:p0 + HD, koh],
                                    start=True, stop=True)
                                pe = ptp.tile([P, NQC], f32r, tag="pe")
                                nc.scalar.activation(pe[:], ps[:], Af.Exp)
                                nc.tensor.matmul(
                                    po[:],
                                    V[:, mt, h * (HD + 1):(h + 1) * (HD + 1)],
                                    pe[:],
                                    start=(mt == 0), stop=(mt == MK // P - 1))
                            rec = ptp.tile([1, NQC], f32r, tag="rec")
                            with nc.allow_low_precision(
                                    reason="f32r softmax denom reciprocal"):
                                nc.vector.reciprocal(rec[0:1, :],
                                                     po[HD:HD + 1, :])
                            p2 = psp.tile([HD, NQC], f32, tag="sm", bufs=2)
                            nc.tensor.matmul(p2[:], ones_r[0:1, :HD],
                                             rec[0:1, :], start=True, stop=True)
                            rb = ptp.tile([HD, NQC], f32, tag="rb")
                            nc.scalar.copy(rb[:], p2[:])
                            nc.vector.tensor_tensor(
                                O_sb[p0:p0 + HD, koh], po[:HD, :], rb[:],
                                Op.mult)

                        # ---- output projection + bias + router weight ----
                        for ko in range(KO):
                            pf = psp.tile([P, NQC], f32, tag="big", bufs=4)
                            for ki in range(KO):
                                nc.tensor.matmul(
                                    pf[:], wo_sb[:, ki, ko * P:(ko + 1) * P],
                                    O_sb[:, ki],
                                    start=(ki == 0), stop=(ki == KO - 1))
                            fin = fpl.tile([P, NQC], f32, tag="fin")
                            nc.vector.tensor_scalar(
                                fin[:], pf[:], bo_sb[:, ko:ko + 1], None, Op.add)
                            nc.vector.tensor_tensor(fin[:], fin[:], w_sb[:],
                                                    Op.mult)
                            nc.sync.dma_start(
                                o_d[b, ko * P:(ko + 1) * P,
                                    tok0:tok0 + NQC], fin[:])
    nc.finalize()
    return nc


def _get_nc():
    global _NC
    if _NC is None:
        _NC = _build_nc()
    return _NC


def kernel(**inputs) -> np.ndarray:
    from concourse.bass_utils import run_bass_kernel_spmd

    ins = {k: np.asarray(v, dtype=np.float32) for k, v in inputs.items()}
    Wr = ins["Wr"]
    br = ins["br"]
    in_maps = []
    for e in range(CORES):
        in_maps.append({
            "q": ins["queries"],
            "k": ins["keys"][:, e * MK:(e + 1) * MK, :],
            "v": ins["values"][:, e * MK:(e + 1) * MK, :],
            "wq": ins["Wq"][e], "wk": ins["Wk"][e],
            "wv": ins["Wv"][e], "wo": ins["Wo"][e],
            "bq": ins["bq"][e], "bk": ins["bk"][e],
            "bv": ins["bv"][e], "bo": ins["bo"][e],
            "wr": np.ascontiguousarray(
                np.concatenate([Wr, Wr[:, e:e + 1]], axis=1)),
            "br": np.ascontiguousarray(
                np.concatenate([br, br[e:e + 1]], axis=0)),
        })
    nc = _get_nc()
    res = run_bass_kernel_spmd(nc, in_maps, list(range(CORES))).results
    acc = res[0]["o"].astype(np.float32)
    for e in range(1, CORES):
        acc = acc + res[e]["o"]
    return np.ascontiguousarray(acc.transpose(0, 2, 1))
